# revision 12
# baseline (speedup 1.0000x reference)
"""DigitCaps u_hat kernel for Trainium2 (8 NeuronCores, SPMD).

Computes u_hat[b,r,c,o] = sum_i W[0,r,c,o,i] * x[b,r,i] + bias[o,0]
with B=512, R=1152, C=10, O=16, I=8 -> output [512, 1152, 10, 16, 1] f32.

Strategy (v2)
-------------
Shard R across the 8 cores: 144 r-values per core; each core writes its
[512, 144, 160] f16 output slice (23.6 MB — the kernel is output-DMA
bound at ~394 GB/s per core, so the whole game is starting that stream
early and never letting it stall).

Per group of G=3 r-values one matmul computes a [128 b, 480] tile:
  k = (r', i), i in [0,9)  (8 x-values + a constant-1 row for the bias)
  lhsT [27, 128] = x^T for a 128-wide b-block      (stationary)
  rhs  [27, 480] = block-diag W (3 x [9,160], bias row included)

v1 zero-padded K 27->128 (K<=32 matmuls stream slower), which required
zeroing every input tile: ~20 us of memset+drain serialized at kernel
start and delayed the first output DMA to t=24 us (trace-measured).
v2 instead uses PE row tiling: 4 groups ("a quad") sit at partition
offsets {0,32,64,96}; each matmul is K=27 in 32x128 tile mode, the four
stream concurrently through disjoint 32-row PE bands into 4 separate
PSUM banks, and the pad rows 27..32 of each band are simply never read
-- no memsets at all.  Inputs land as one contiguous [128, 2, 992] f16
DMA per chunk (full 16-port coverage vs 27-partition transfers in v1).

PSUM->SBUF evacuation alternates DVE/ACT per quad ([128,4,480] f32->f16
copies); output DMAs (2 quads = 0.98 MB) stream on the sync HWDGE ring.
"""

import numpy as np

# Problem constants (hardcoded per harness contract).
B, R, C, O, I = 512, 1152, 10, 16, 8
CO = C * O                      # 160
NCORES = 8
RS = R // NCORES                # 144 r per core
G = 3                           # r-values per matmul (block-diag pack)
K = G * (I + 1)                 # 27 contraction rows (incl. bias row)
BANDS = 4                       # row-tiled matmuls per quad (PE 32x128 mode)
QUADS = RS // (G * BANDS)       # 12 quads per core
QPC = 2                         # quads per input chunk
CHUNKS = QUADS // QPC           # 6 input chunks (for early compute start)
XC = B                          # 512 x columns per quad slot
WC = G * CO                     # 480 W columns per quad slot
TC = XC + WC                    # 992 packed input columns
DMA_Q = 2                       # quads per output DMA (~1 MB transfers)
BBLK = B // 128                 # 4 b-blocks

OP_DT = "f16"                   # operand dtype (kept for test.py compat)
OUT_DT = "f16"                  # device output dtype

_prog_cache = {}


def _build_program(op_dt=OP_DT, out_dt=OUT_DT):
    import concourse.bacc as bacc
    import concourse.tile as tile
    from concourse import mybir

    key = (op_dt, out_dt)
    if key in _prog_cache:
        return _prog_cache[key]

    f32 = mybir.dt.float32
    f16 = mybir.dt.float16

    # Bacc (not raw Bass): its finalize() runs move_matmul_waits_to_ldweights
    # + generate_event_semaphores, required to satisfy the per-instruction
    # sync-wait limits at codegen.
    nc = bacc.Bacc("TRN2", target_bir_lowering=False, debug=False)

    in_d = nc.declare_dram_parameter(
        "inp", [128, QUADS, TC], f16, isOutput=False
    )
    out_d = nc.declare_dram_parameter("out", [B, RS, CO], f16, isOutput=True)

    with tile.TileContext(nc) as tc:
        with (
            tc.tile_pool(name="const", bufs=1) as const,
            tc.tile_pool(name="psum", bufs=2, space="PSUM") as psum,
            tc.tile_pool(name="outp", bufs=4) as outp,
        ):
            # One flat input tile, loaded in 4 DMAs on the scalar ring
            # (q10) -- a queue neither output stream uses, so the input
            # backlog can never queue ahead of output packets.  Quad 0/1
            # pilots land first (FIFO), then two consolidated transfers
            # whose big per-partition descriptors run near line rate.
            insb = const.tile([128, QUADS, TC], f16, tag="insb")
            nc.scalar.dma_start(out=insb[:, 0], in_=in_d[:, 0])
            nc.scalar.dma_start(out=insb[:, 1], in_=in_d[:, 1])
            nc.scalar.dma_start(out=insb[:, 2:6], in_=in_d[:, 2:6])
            nc.scalar.dma_start(out=insb[:, 6:12], in_=in_d[:, 6:12])

            for j in range(BBLK):
                for dq in range(QUADS // DMA_Q):
                    ot = outp.tile([128, DMA_Q, BANDS, WC], f16)
                    for s2 in range(DMA_Q):
                        q = dq * DMA_Q + s2
                        # Two 2-bank psum tiles per quad with DEDICATED tag
                        # rings (bufs=2 each): tile reuse chains A(m)->A(m-2)
                        # give two quads of slack, so neither the matmuls nor
                        # the copies ever wait on the previous quad (the
                        # untagged pool interleaves A/B in one ring, which
                        # serializes quad m's matmuls behind quad m-1's
                        # copies -- measured 1.47us/quad vs the 1.25us DMA
                        # floor).
                        for h in range(2):
                            ps = psum.tile(
                                [128, 2, 512], f32, tag=f"ps{h}", bufs=2
                            )
                            for b2 in range(2):
                                band = 2 * h + b2
                                pb = 32 * band
                                lhsT = insb[
                                    pb : pb + K, q, j * 128 : (j + 1) * 128
                                ]
                                rhs = insb[pb : pb + K, q, XC : XC + WC]
                                # K=27 at partition offset pb: 32x128
                                # row-tile mode, 4 concurrent streams into 4
                                # banks.  Explicit tile_position: auto-derive
                                # rejects base partition 96.
                                nc.tensor.matmul(
                                    ps[:, b2, 0:WC], lhsT, rhs,
                                    start=True, stop=True,
                                    tile_position=(pb, 0),
                                )
                            if h == 0:
                                nc.vector.tensor_copy(
                                    ot[:, s2, 0:2, :], ps[:, :, 0:WC]
                                )
                            else:
                                nc.scalar.copy(
                                    ot[:, s2, 2:4, :], ps[:, :, 0:WC]
                                )
                    # Alternate output DMAs across two queues (sync HWDGE
                    # and gpsimd SWDGE): while one queue's engine slot
                    # waits on a transfer's completion receipt, the other
                    # queue's packets keep the SDMA engines busy.
                    oeng = nc.sync if (j * (QUADS // DMA_Q) + dq) % 2 == 0 else nc.gpsimd
                    oeng.dma_start(
                        out=out_d[
                            j * 128 : (j + 1) * 128,
                            dq * DMA_Q * G * BANDS : (dq + 1) * DMA_Q * G * BANDS,
                            :,
                        ],
                        in_=ot[:],
                    )

    nc.finalize()
    _prog_cache[key] = nc
    return nc


def _prep_inputs(x, W, bias, op_dt=OP_DT):
    """Build per-core packed input arrays in the device layout.

    Per chunk: [128, QPC, TC] f16 where partition p = 32*band + k,
    k = r'*9 + i (i=8 is the constant-1 bias row; rows 27..32 are pad),
    slot s picks the quad, cols [0:512] = x^T (b), cols [512:992] = the
    [27, 480] block-diag W for the band's group.
    """
    x = np.ascontiguousarray(x, dtype=np.float32)
    W = np.ascontiguousarray(W, dtype=np.float32)
    bias = np.ascontiguousarray(bias, dtype=np.float32)

    Wf = W[0].reshape(R, CO, I)                      # [R, CO, I]
    bias_co = np.tile(bias[:, 0], C)                 # [CO]
    NG = RS // G                                     # 48 groups per core

    in_maps = []
    for c in range(NCORES):
        r0 = c * RS
        arr = np.zeros((BANDS, 32, QUADS, TC), dtype=np.float16)

        xT = x[:, r0 : r0 + RS, :].transpose(1, 2, 0)    # [RS, I, B]
        seg9 = np.empty((RS, I + 1, B), dtype=np.float32)
        seg9[:, :I] = xT
        seg9[:, I] = 1.0
        g27 = seg9.reshape(NG, K, B)                     # rows k = r'*9+i
        # group g = q*BANDS + band
        arr[:, :K, :, :XC] = (
            g27.reshape(QUADS, BANDS, K, B).transpose(1, 2, 0, 3)
        )

        Wc = Wf[r0 : r0 + RS]                            # [RS, CO, I]
        W9 = np.empty((RS, I + 1, CO), dtype=np.float32)
        W9[:, :I] = Wc.transpose(0, 2, 1)
        W9[:, I] = bias_co
        blk = np.zeros((NG, G, I + 1, G, CO), dtype=np.float32)
        W9g = W9.reshape(NG, G, I + 1, CO)
        for rp in range(G):
            blk[:, rp, :, rp, :] = W9g[:, rp]
        blk27 = blk.reshape(NG, K, WC)
        arr[:, :K, :, XC:] = (
            blk27.reshape(QUADS, BANDS, K, WC).transpose(1, 2, 0, 3)
        )

        in_maps.append({"inp": arr.reshape(128, QUADS, TC)})
    return in_maps


def _run(inputs, trace=False, op_dt=OP_DT, out_dt=OUT_DT, **kw):
    from concourse.bass_utils import run_bass_kernel_spmd

    nc = _build_program(op_dt, out_dt)
    in_maps = _prep_inputs(inputs["x"], inputs["W"], inputs["bias"], op_dt)
    res = run_bass_kernel_spmd(
        nc, in_maps, list(range(NCORES)), trace=trace, **kw
    )
    outs = [np.asarray(res.results[c]["out"]) for c in range(NCORES)]
    full = np.concatenate(outs, axis=1)               # [B, R, CO]
    full = full.astype(np.float32, copy=False)
    return np.ascontiguousarray(full).reshape(B, R, C, O, 1), res


def kernel(x, W, bias):
    out, _ = _run({"x": x, "W": W, "bias": bias})
    return out


# revision 14
# speedup vs baseline: 1.1292x; 1.1292x over previous
"""DigitCaps u_hat kernel for Trainium2 (8 NeuronCores, SPMD).

Computes u_hat[b,r,c,o] = sum_i W[0,r,c,o,i] * x[b,r,i] + bias[o,0]
with B=512, R=1152, C=10, O=16, I=8 -> output [512, 1152, 10, 16, 1] f32.

Strategy (v2)
-------------
Shard R across the 8 cores: 144 r-values per core; each core writes its
[512, 144, 160] f16 output slice (23.6 MB — the kernel is output-DMA
bound at ~394 GB/s per core, so the whole game is starting that stream
early and never letting it stall).

Per group of G=3 r-values one matmul computes a [128 b, 480] tile:
  k = (r', i), i in [0,9)  (8 x-values + a constant-1 row for the bias)
  lhsT [27, 128] = x^T for a 128-wide b-block      (stationary)
  rhs  [27, 480] = block-diag W (3 x [9,160], bias row included)

v1 zero-padded K 27->128 (K<=32 matmuls stream slower), which required
zeroing every input tile: ~20 us of memset+drain serialized at kernel
start and delayed the first output DMA to t=24 us (trace-measured).
v2 instead uses PE row tiling: 4 groups ("a quad") sit at partition
offsets {0,32,64,96}; each matmul is K=27 in 32x128 tile mode, the four
stream concurrently through disjoint 32-row PE bands into 4 separate
PSUM banks, and the pad rows 27..32 of each band are simply never read
-- no memsets at all.  Inputs land as one contiguous [128, 2, 992] f16
DMA per chunk (full 16-port coverage vs 27-partition transfers in v1).

PSUM->SBUF evacuation alternates DVE/ACT per quad ([128,4,480] f32->f16
copies); output DMAs (2 quads = 0.98 MB) stream on the sync HWDGE ring.
"""

import numpy as np

# Problem constants (hardcoded per harness contract).
B, R, C, O, I = 512, 1152, 10, 16, 8
CO = C * O                      # 160
NCORES = 8
RS = R // NCORES                # 144 r per core
G = 3                           # r-values per matmul (block-diag pack)
K = G * (I + 1)                 # 27 contraction rows (incl. bias row)
BANDS = 4                       # row-tiled matmuls per quad (PE 32x128 mode)
QUADS = RS // (G * BANDS)       # 12 quads per core
QPC = 2                         # quads per input chunk
CHUNKS = QUADS // QPC           # 6 input chunks (for early compute start)
XC = B                          # 512 x columns per quad slot
WC = G * CO                     # 480 W columns per quad slot
TC = XC + WC                    # 992 packed input columns
DMA_Q = 2                       # quads per output DMA (~1 MB transfers)
BBLK = B // 128                 # 4 b-blocks

OP_DT = "f16"                   # operand dtype (kept for test.py compat)
OUT_DT = "f16"                  # device output dtype

_prog_cache = {}


def _build_program(op_dt=OP_DT, out_dt=OUT_DT):
    import concourse.bacc as bacc
    import concourse.tile as tile
    from concourse import mybir

    key = (op_dt, out_dt)
    if key in _prog_cache:
        return _prog_cache[key]

    f32 = mybir.dt.float32
    f16 = mybir.dt.float16

    # Bacc (not raw Bass): its finalize() runs move_matmul_waits_to_ldweights
    # + generate_event_semaphores, required to satisfy the per-instruction
    # sync-wait limits at codegen.
    nc = bacc.Bacc("TRN2", target_bir_lowering=False, debug=False)

    in_d = nc.declare_dram_parameter(
        "inp", [128, QUADS, TC], f16, isOutput=False
    )
    out_d = nc.declare_dram_parameter("out", [B, RS, CO], f16, isOutput=True)

    with tile.TileContext(nc) as tc:
        with (
            tc.tile_pool(name="const", bufs=1) as const,
            tc.tile_pool(name="psum", bufs=2, space="PSUM") as psum,
            tc.tile_pool(name="outp", bufs=6) as outp,
        ):
            # One flat input tile, loaded in 4 DMAs on the scalar ring
            # (q10) -- a queue neither output stream uses, so the input
            # backlog can never queue ahead of output packets.  Quad 0/1
            # pilots land first (FIFO), then two consolidated transfers
            # whose big per-partition descriptors run near line rate.
            insb = const.tile([128, QUADS, TC], f16, tag="insb")
            nc.scalar.dma_start(out=insb[:, 0], in_=in_d[:, 0])
            nc.scalar.dma_start(out=insb[:, 1], in_=in_d[:, 1])
            nc.scalar.dma_start(out=insb[:, 2:6], in_=in_d[:, 2:6])
            nc.scalar.dma_start(out=insb[:, 6:12], in_=in_d[:, 6:12])

            for j in range(BBLK):
                for dq in range(QUADS // DMA_Q):
                    ot = outp.tile([128, DMA_Q, BANDS, WC], f16)
                    for s2 in range(DMA_Q):
                        q = dq * DMA_Q + s2
                        # Two 2-bank psum tiles per quad with DEDICATED tag
                        # rings (bufs=2 each): tile reuse chains A(m)->A(m-2)
                        # give two quads of slack, so neither the matmuls nor
                        # the copies ever wait on the previous quad (the
                        # untagged pool interleaves A/B in one ring, which
                        # serializes quad m's matmuls behind quad m-1's
                        # copies -- measured 1.47us/quad vs the 1.25us DMA
                        # floor).
                        for h in range(2):
                            ps = psum.tile(
                                [128, 2, 512], f32, tag=f"ps{h}", bufs=2
                            )
                            for b2 in range(2):
                                band = 2 * h + b2
                                pb = 32 * band
                                lhsT = insb[
                                    pb : pb + K, q, j * 128 : (j + 1) * 128
                                ]
                                rhs = insb[pb : pb + K, q, XC : XC + WC]
                                # K=27 at partition offset pb: 32x128
                                # row-tile mode, 4 concurrent streams into 4
                                # banks.  Explicit tile_position: auto-derive
                                # rejects base partition 96.
                                nc.tensor.matmul(
                                    ps[:, b2, 0:WC], lhsT, rhs,
                                    start=True, stop=True,
                                    tile_position=(pb, 0),
                                )
                            if h == 0:
                                nc.vector.tensor_copy(
                                    ot[:, s2, 0:2, :], ps[:, :, 0:WC]
                                )
                            else:
                                nc.scalar.copy(
                                    ot[:, s2, 2:4, :], ps[:, :, 0:WC]
                                )
                    # All outputs on the sync ring (q1).  SDMA arbitration
                    # is strict-priority by queue index (q0>q1>q10,
                    # measured): outputs on q1 preempt the input tail on
                    # q10, while splitting outputs across q0+q1 ping-pongs
                    # (measured 106us).
                    nc.sync.dma_start(
                        out=out_d[
                            j * 128 : (j + 1) * 128,
                            dq * DMA_Q * G * BANDS : (dq + 1) * DMA_Q * G * BANDS,
                            :,
                        ],
                        in_=ot[:],
                    )

    nc.finalize()
    _prog_cache[key] = nc
    return nc


def _prep_inputs(x, W, bias, op_dt=OP_DT):
    """Build per-core packed input arrays in the device layout.

    Per chunk: [128, QPC, TC] f16 where partition p = 32*band + k,
    k = r'*9 + i (i=8 is the constant-1 bias row; rows 27..32 are pad),
    slot s picks the quad, cols [0:512] = x^T (b), cols [512:992] = the
    [27, 480] block-diag W for the band's group.
    """
    x = np.ascontiguousarray(x, dtype=np.float32)
    W = np.ascontiguousarray(W, dtype=np.float32)
    bias = np.ascontiguousarray(bias, dtype=np.float32)

    Wf = W[0].reshape(R, CO, I)                      # [R, CO, I]
    bias_co = np.tile(bias[:, 0], C)                 # [CO]
    NG = RS // G                                     # 48 groups per core

    in_maps = []
    for c in range(NCORES):
        r0 = c * RS
        arr = np.zeros((BANDS, 32, QUADS, TC), dtype=np.float16)

        xT = x[:, r0 : r0 + RS, :].transpose(1, 2, 0)    # [RS, I, B]
        seg9 = np.empty((RS, I + 1, B), dtype=np.float32)
        seg9[:, :I] = xT
        seg9[:, I] = 1.0
        g27 = seg9.reshape(NG, K, B)                     # rows k = r'*9+i
        # group g = q*BANDS + band
        arr[:, :K, :, :XC] = (
            g27.reshape(QUADS, BANDS, K, B).transpose(1, 2, 0, 3)
        )

        Wc = Wf[r0 : r0 + RS]                            # [RS, CO, I]
        W9 = np.empty((RS, I + 1, CO), dtype=np.float32)
        W9[:, :I] = Wc.transpose(0, 2, 1)
        W9[:, I] = bias_co
        blk = np.zeros((NG, G, I + 1, G, CO), dtype=np.float32)
        W9g = W9.reshape(NG, G, I + 1, CO)
        for rp in range(G):
            blk[:, rp, :, rp, :] = W9g[:, rp]
        blk27 = blk.reshape(NG, K, WC)
        arr[:, :K, :, XC:] = (
            blk27.reshape(QUADS, BANDS, K, WC).transpose(1, 2, 0, 3)
        )

        in_maps.append({"inp": arr.reshape(128, QUADS, TC)})
    return in_maps


def _run(inputs, trace=False, op_dt=OP_DT, out_dt=OUT_DT, **kw):
    from concourse.bass_utils import run_bass_kernel_spmd

    nc = _build_program(op_dt, out_dt)
    in_maps = _prep_inputs(inputs["x"], inputs["W"], inputs["bias"], op_dt)
    res = run_bass_kernel_spmd(
        nc, in_maps, list(range(NCORES)), trace=trace, **kw
    )
    outs = [np.asarray(res.results[c]["out"]) for c in range(NCORES)]
    full = np.concatenate(outs, axis=1)               # [B, R, CO]
    full = full.astype(np.float32, copy=False)
    return np.ascontiguousarray(full).reshape(B, R, C, O, 1), res


def kernel(x, W, bias):
    out, _ = _run({"x": x, "W": W, "bias": bias})
    return out


# revision 22
# speedup vs baseline: 1.2344x; 1.0932x over previous
"""DigitCaps u_hat kernel for Trainium2 (8 NeuronCores, SPMD).

Computes u_hat[b,r,c,o] = sum_i W[0,r,c,o,i] * x[b,r,i] + bias[o,0]
with B=512, R=1152, C=10, O=16, I=8 -> output [512, 1152, 10, 16, 1] f32.

Strategy (v2)
-------------
Shard R across the 8 cores: 144 r-values per core; each core writes its
[512, 144, 160] f16 output slice (23.6 MB — the kernel is output-DMA
bound at ~394 GB/s per core, so the whole game is starting that stream
early and never letting it stall).

Per group of G=3 r-values one matmul computes a [128 b, 480] tile:
  k = (r', i), i in [0,9)  (8 x-values + a constant-1 row for the bias)
  lhsT [27, 128] = x^T for a 128-wide b-block      (stationary)
  rhs  [27, 480] = block-diag W (3 x [9,160], bias row included)

v1 zero-padded K 27->128 (K<=32 matmuls stream slower), which required
zeroing every input tile: ~20 us of memset+drain serialized at kernel
start and delayed the first output DMA to t=24 us (trace-measured).
v2 instead uses PE row tiling: 4 groups ("a quad") sit at partition
offsets {0,32,64,96}; each matmul is K=27 in 32x128 tile mode, the four
stream concurrently through disjoint 32-row PE bands into 4 separate
PSUM banks, and the pad rows 27..32 of each band are simply never read
-- no memsets at all.  Inputs land as one contiguous [128, 2, 992] f16
DMA per chunk (full 16-port coverage vs 27-partition transfers in v1).

PSUM->SBUF evacuation alternates DVE/ACT per quad ([128,4,480] f32->f16
copies); output DMAs (2 quads = 0.98 MB) stream on the sync HWDGE ring.
"""

import numpy as np

# Problem constants (hardcoded per harness contract).
B, R, C, O, I = 512, 1152, 10, 16, 8
CO = C * O                      # 160
NCORES = 8
RS = R // NCORES                # 144 r per core
G = 3                           # r-values per matmul (block-diag pack)
K = G * (I + 1)                 # 27 contraction rows (incl. bias row)
BANDS = 4                       # row-tiled matmuls per quad (PE 32x128 mode)
QUADS = RS // (G * BANDS)       # 12 quads per core
QPC = 2                         # quads per input chunk
CHUNKS = QUADS // QPC           # 6 input chunks (for early compute start)
XC = B                          # 512 x columns per quad slot
WC = G * CO                     # 480 W columns per quad slot
TC = XC + WC                    # 992 packed input columns
DMA_Q = 2                       # quads per output DMA (~1 MB transfers)
BBLK = B // 128                 # 4 b-blocks

OP_DT = "f16"                   # operand dtype (kept for test.py compat)
OUT_DT = "f16"                  # device output dtype

_prog_cache = {}


def _build_program(op_dt=OP_DT, out_dt=OUT_DT):
    import concourse.bacc as bacc
    import concourse.tile as tile
    from concourse import mybir

    key = (op_dt, out_dt)
    if key in _prog_cache:
        return _prog_cache[key]

    f32 = mybir.dt.float32
    f16 = mybir.dt.float16

    # Bacc (not raw Bass): its finalize() runs move_matmul_waits_to_ldweights
    # + generate_event_semaphores, required to satisfy the per-instruction
    # sync-wait limits at codegen.
    nc = bacc.Bacc("TRN2", target_bir_lowering=False, debug=False)

    xp_d = nc.declare_dram_parameter(
        "xp", [128, QUADS, XC], f16, isOutput=False
    )
    wc_d = nc.declare_dram_parameter(
        "wc", [BANDS, G, I + 1, QUADS, CO], f16, isOutput=False
    )
    out_d = nc.declare_dram_parameter("out", [B, RS, CO], f16, isOutput=True)

    with tile.TileContext(nc) as tc:
        with (
            tc.tile_pool(name="const", bufs=1) as const,
            tc.tile_pool(name="psum", bufs=2, space="PSUM") as psum,
            tc.tile_pool(name="outp", bufs=6) as outp,
        ):
            # Input is 1.97 MB: x padded to the 32-row bands (1.57 MB,
            # chunked contiguous DMAs on the gpsimd queue, quad 0 first),
            # plus COMPACT W (0.40 MB): one dense DVE memset zeroes wt
            # (4x mode, done by ~8.3us, long before the first CAST needs
            # the engine), then 12 [9-partition] DMAs place the non-zero
            # blocks of each band's block-diagonal, split across the
            # sync+scalar rings which are otherwise idle until the first
            # output/copy.  (Multi-level partition APs in one DMA would
            # do this in 3 transfers but silently mis-lower - measured.)
            xt = const.tile([128, QUADS, XC], f16, tag="xt")
            wt = const.tile([128, QUADS, WC], f16, tag="wt")
            nc.vector.memset(wt[:], 0)
            for band in range(BANDS):
                for rp in range(G):
                    p0 = 32 * band + 9 * rp
                    eng = nc.sync if band < 2 else nc.scalar
                    eng.dma_start(
                        out=wt[p0 : p0 + 9, :, rp * CO : (rp + 1) * CO],
                        in_=wc_d[band, rp],
                    )
            nc.gpsimd.dma_start(out=xt[:, 0:1], in_=xp_d[:, 0:1])
            nc.gpsimd.dma_start(out=xt[:, 1:2], in_=xp_d[:, 1:2])
            nc.gpsimd.dma_start(out=xt[:, 2:4], in_=xp_d[:, 2:4])
            nc.gpsimd.dma_start(out=xt[:, 4:6], in_=xp_d[:, 4:6])
            nc.gpsimd.dma_start(out=xt[:, 6:9], in_=xp_d[:, 6:9])
            nc.gpsimd.dma_start(out=xt[:, 9:12], in_=xp_d[:, 9:12])

            for j in range(BBLK):
                for dq in range(QUADS // DMA_Q):
                    ot = outp.tile([128, DMA_Q, BANDS, WC], f16)
                    for s2 in range(DMA_Q):
                        q = dq * DMA_Q + s2
                        # Two 2-bank psum tiles per quad with DEDICATED tag
                        # rings (bufs=2 each): tile reuse chains A(m)->A(m-2)
                        # give two quads of slack, so neither the matmuls nor
                        # the copies ever wait on the previous quad (the
                        # untagged pool interleaves A/B in one ring, which
                        # serializes quad m's matmuls behind quad m-1's
                        # copies -- measured 1.47us/quad vs the 1.25us DMA
                        # floor).
                        for h in range(2):
                            ps = psum.tile(
                                [128, 2, 512], f32, tag=f"ps{h}", bufs=2
                            )
                            for b2 in range(2):
                                band = 2 * h + b2
                                pb = 32 * band
                                lhsT = xt[
                                    pb : pb + K, q, j * 128 : (j + 1) * 128
                                ]
                                rhs = wt[pb : pb + K, q, :]
                                # K=27 at partition offset pb: 32x128
                                # row-tile mode, 4 concurrent streams into 4
                                # banks.  Explicit tile_position: auto-derive
                                # rejects base partition 96.
                                nc.tensor.matmul(
                                    ps[:, b2, 0:WC], lhsT, rhs,
                                    start=True, stop=True,
                                    tile_position=(pb, 0),
                                )
                            if h == 0:
                                nc.vector.tensor_copy(
                                    ot[:, s2, 0:2, :], ps[:, :, 0:WC]
                                )
                            else:
                                nc.scalar.copy(
                                    ot[:, s2, 2:4, :], ps[:, :, 0:WC]
                                )
                    # All outputs on the sync ring (q1).  SDMA arbitration
                    # is strict-priority by queue index (q0>q1>q10,
                    # measured): outputs on q1 preempt the input tail on
                    # q10, while splitting outputs across q0+q1 ping-pongs
                    # (measured 106us).
                    nc.sync.dma_start(
                        out=out_d[
                            j * 128 : (j + 1) * 128,
                            dq * DMA_Q * G * BANDS : (dq + 1) * DMA_Q * G * BANDS,
                            :,
                        ],
                        in_=ot[:],
                    )

    nc.finalize()
    _prog_cache[key] = nc
    return nc


def _prep_inputs(x, W, bias, op_dt=OP_DT):
    """Build per-core packed input arrays in the device layout.

    Per chunk: [128, QPC, TC] f16 where partition p = 32*band + k,
    k = r'*9 + i (i=8 is the constant-1 bias row; rows 27..32 are pad),
    slot s picks the quad, cols [0:512] = x^T (b), cols [512:992] = the
    [27, 480] block-diag W for the band's group.
    """
    x = np.ascontiguousarray(x, dtype=np.float32)
    W = np.ascontiguousarray(W, dtype=np.float32)
    bias = np.ascontiguousarray(bias, dtype=np.float32)

    Wf = W[0].reshape(R, CO, I)                      # [R, CO, I]
    bias_co = np.tile(bias[:, 0], C)                 # [CO]
    NG = RS // G                                     # 48 groups per core

    in_maps = []
    for c in range(NCORES):
        r0 = c * RS

        xT = x[:, r0 : r0 + RS, :].transpose(1, 2, 0)    # [RS, I, B]
        seg9 = np.empty((RS, I + 1, B), dtype=np.float32)
        seg9[:, :I] = xT
        seg9[:, I] = 1.0
        g27 = seg9.reshape(NG, K, B)                     # rows k = r'*9+i
        # group g = q*BANDS + band; x padded into [BANDS, 32, ...] bands
        xp = np.zeros((BANDS, 32, QUADS, B), dtype=np.float16)
        xp[:, :K] = g27.reshape(QUADS, BANDS, K, B).transpose(1, 2, 0, 3)

        Wc = Wf[r0 : r0 + RS]                            # [RS, CO, I]
        W9 = np.empty((RS, I + 1, CO), dtype=np.float32)
        W9[:, :I] = Wc.transpose(0, 2, 1)
        W9[:, I] = bias_co
        W9g = W9.reshape(QUADS, BANDS, G, I + 1, CO)
        wc = np.ascontiguousarray(
            W9g.transpose(1, 2, 3, 0, 4)
        ).astype(np.float16)                             # [BANDS, G, 9, QUADS, CO]

        in_maps.append({"xp": xp.reshape(128, QUADS, B), "wc": wc})
    return in_maps


def _run(inputs, trace=False, op_dt=OP_DT, out_dt=OUT_DT, **kw):
    from concourse.bass_utils import run_bass_kernel_spmd

    nc = _build_program(op_dt, out_dt)
    in_maps = _prep_inputs(inputs["x"], inputs["W"], inputs["bias"], op_dt)
    res = run_bass_kernel_spmd(
        nc, in_maps, list(range(NCORES)), trace=trace, **kw
    )
    outs = [np.asarray(res.results[c]["out"]) for c in range(NCORES)]
    full = np.concatenate(outs, axis=1)               # [B, R, CO]
    full = full.astype(np.float32, copy=False)
    return np.ascontiguousarray(full).reshape(B, R, C, O, 1), res


def kernel(x, W, bias):
    out, _ = _run({"x": x, "W": W, "bias": bias})
    return out
